# revision 1
# baseline (speedup 1.0000x reference)
"""Trainium2 Bass kernel for the ConditionalDETR sparse-key (topk masking) block.

Computation (per batch image b):
  cls    = outputs_class[b].max(-1)                       # (300,)
  sel    = top-150 of cls (stable, set semantics)         # (300,) 0/1
  boxes  -> pixel xyxy via img_true_sizes[b]
  m[p]   = not (grid point (16i,16j) inside any selected box) | pad[p]   # p = i*32+j
  d[p]   = exclusive prefix sum of m  (destination row for kept tokens)
  out[d[p], b, :] = x[b, :, p]  for m[p]=1 ; remaining rows = 0

Sharding: 8 cores = 4 batches x 2 channel halves (128 ch each); pure data
parallel, identical program on every core (SPMD).

On-device implementation highlights:
  - top-k selection via stable-rank = #{j: cls_j > cls_i} + #{j<i: cls_j == cls_i}
    computed with an all-pairs compare matrix (exact fp32, matches
    jax.lax.top_k tie semantics).
  - point-in-box mask via separable interval masks X^T (q,32) / Y^T (q,32)
    and one PE matmul S = Y^T.T @ X^T (counts; exact small integers).
  - prefix sums via strict-triangular matmuls.
  - permutation applied with one indirect-DMA scatter per tensor; masked-out
    rows get dest=4000 and are dropped by bounds_check (output buffers are
    pre-zeroed by the runner, so dropped rows stay exactly 0).
"""

import sys

import numpy as np

if "/opt/trn_rl_repo" not in sys.path:
    sys.path.insert(0, "/opt/trn_rl_repo")

BS, C, H, W = 4, 256, 32, 32
HW = H * W          # 1024
NQ, NCLS = 300, 80
TOPK = 150
CH = 128            # channels per core
NCORES = 8
CHUNKS = [128, 128, 44]   # 300 queries in partition chunks
NT = HW // 128      # 8 column tiles of x per core

_cache = {}


def _emit(tc, bass, mybir):
    from concourse.masks import make_identity

    nc = tc.nc
    f32 = mybir.dt.float32
    i32 = mybir.dt.int32
    u8 = mybir.dt.uint8
    Alu = mybir.AluOpType
    AX = mybir.AxisListType

    io = _cache["io"]

    with tc.tile_pool(name="sb", bufs=1) as sb, \
         tc.tile_pool(name="ps", bufs=1, space="PSUM") as ps, \
         tc.tile_pool(name="dr", bufs=1, space="DRAM") as dr:

        # ---------------- constants (built on device) ----------------
        ident = sb.tile([128, 128], f32, name="ident")
        make_identity(nc, ident[:])

        g16i = sb.tile([128, 32], i32, name="g16i")
        nc.gpsimd.iota(g16i[:], pattern=[[16, 32]], base=0, channel_multiplier=0)
        g16 = sb.tile([128, 32], f32, name="g16")
        nc.vector.tensor_copy(out=g16[:], in_=g16i[:])

        # T32[a, b] = 1.0 iff a < b  (strict upper triangular)
        T32 = sb.tile([32, 32], f32, name="T32")
        nc.gpsimd.memset(T32[:], 1.0)
        nc.gpsimd.affine_select(
            out=T32[:], in_=T32[:], compare_op=Alu.is_gt, fill=0.0,
            base=0, channel_multiplier=-1, pattern=[[1, 32]])

        # LT[k][p, j] = 1.0 iff j < 128k + p (stable tie-break masks)
        LT = []
        for k, n in enumerate(CHUNKS):
            t = sb.tile([128, NQ], f32, name=f"LT{k}")
            nc.gpsimd.memset(t[:n], 1.0)
            nc.gpsimd.affine_select(
                out=t[:n], in_=t[:n], compare_op=Alu.is_gt, fill=0.0,
                base=128 * k, channel_multiplier=1, pattern=[[-1, NQ]])
            LT.append(t)

        # ---------------- input loads ----------------
        # small latency-critical loads lead the SP HWDGE FIFO; xh follows;
        # ph rides the ACT HWDGE ring so both big loads go in parallel.
        CLS = []
        CRD = []
        for k, n in enumerate(CHUNKS):
            t = sb.tile([128, NCLS], f32, name=f"CLS{k}")
            nc.sync.dma_start(out=t[:n], in_=io["cls"][128 * k:128 * k + n, :])
            CLS.append(t)
            t = sb.tile([128, 4], f32, name=f"CRD{k}")
            nc.sync.dma_start(out=t[:n], in_=io["crd"][128 * k:128 * k + n, :])
            CRD.append(t)

        TSZ = sb.tile([1, 2], i32, name="TSZ")
        nc.sync.dma_start(out=TSZ[:1], in_=io["tsz"])
        PAD8 = sb.tile([32, 32], u8, name="PAD8")
        nc.sync.dma_start(out=PAD8[:32], in_=io["pmask"])

        XH = sb.tile([128, HW], f32, name="XH")
        nc.sync.dma_start(out=XH[:], in_=io["xh"])
        PH = sb.tile([128, HW], f32, name="PH")
        nc.scalar.dma_start(out=PH[:], in_=io["ph"])

        # ---------------- cls max + transpose + broadcast ----------------
        ccol = []
        for k, n in enumerate(CHUNKS):
            t = sb.tile([128, 1], f32, name=f"ccol{k}")
            nc.vector.tensor_reduce(t[:n], CLS[k][:n, :], axis=AX.X, op=Alu.max)
            ccol.append(t)

        CROW = sb.tile([1, NQ], f32, name="CROW")
        for k, n in enumerate(CHUNKS):
            crps = ps.tile([1, 128], f32, tag="crps", bufs=1)
            nc.tensor.transpose(out=crps[:1, :n], in_=ccol[k][:n, :1],
                                identity=ident[:n, :n])
            nc.vector.tensor_copy(out=CROW[:1, 128 * k:128 * k + n],
                                  in_=crps[:1, :n])

        CBC = sb.tile([128, NQ], f32, name="CBC")
        nc.gpsimd.partition_broadcast(CBC[:], CROW[:1, :])

        # img_true_sizes -> f32, broadcast across partitions
        TSF = sb.tile([1, 2], f32, name="TSF")
        nc.vector.tensor_copy(out=TSF[:1], in_=TSZ[:1])
        TSB = sb.tile([128, 2], f32, name="TSB")
        nc.gpsimd.partition_broadcast(TSB[:], TSF[:1, :])

        # ---------------- per-chunk: rank/sel, boxes, X/Y masks, S ----------------
        S32 = ps.tile([32, 32], f32, tag="S32")
        for k, n in enumerate(CHUNKS):
            # stable rank of each query's cls among all 300:
            #   rank = #{j: cls_j > cls_i} + #{j < i: cls_j == cls_i}
            G = sb.tile([128, NQ], f32, tag="G", bufs=2)
            rankG = sb.tile([128, 1], f32, tag="rankG", bufs=2)
            nc.vector.tensor_scalar(out=G[:n], in0=CBC[:n],
                                    scalar1=ccol[k][:n, 0:1], scalar2=None,
                                    op0=Alu.is_gt, op1=Alu.add,
                                    accum_out=rankG[:n])
            E = sb.tile([128, NQ], f32, tag="E", bufs=2)
            rankE = sb.tile([128, 1], f32, tag="rankE", bufs=2)
            nc.vector.scalar_tensor_tensor(
                out=E[:n], in0=CBC[:n], scalar=ccol[k][:n, 0:1], in1=LT[k][:n],
                op0=Alu.is_equal, op1=Alu.mult, accum_out=rankE[:n])
            rank = sb.tile([128, 1], f32, tag="rank", bufs=2)
            nc.vector.tensor_tensor(out=rank[:n], in0=rankG[:n], in1=rankE[:n],
                                    op=Alu.add)
            sel = sb.tile([128, 1], f32, tag="sel", bufs=2)
            nc.vector.tensor_scalar(out=sel[:n], in0=rank[:n],
                                    scalar1=float(TOPK), scalar2=None,
                                    op0=Alu.is_lt)

            # boxes -> scaled xyxy on GPSIMD, concurrent with the DVE rank ops
            # (identical fp32 op order as the reference: sub/add then mult)
            crd = CRD[k]
            w05 = sb.tile([128, 1], f32, tag="w05", bufs=2)
            nc.vector.tensor_scalar(out=w05[:n], in0=crd[:n, 2:3],
                                    scalar1=0.5, scalar2=None, op0=Alu.mult)
            h05 = sb.tile([128, 1], f32, tag="h05", bufs=2)
            nc.vector.tensor_scalar(out=h05[:n], in0=crd[:n, 3:4],
                                    scalar1=0.5, scalar2=None, op0=Alu.mult)

            x1 = sb.tile([128, 1], f32, tag="x1", bufs=2)
            nc.vector.scalar_tensor_tensor(
                out=x1[:n], in0=crd[:n, 0:1], scalar=w05[:n, 0:1],
                in1=TSB[:n, 0:1], op0=Alu.subtract, op1=Alu.mult)
            x2 = sb.tile([128, 1], f32, tag="x2", bufs=2)
            nc.vector.scalar_tensor_tensor(
                out=x2[:n], in0=crd[:n, 0:1], scalar=w05[:n, 0:1],
                in1=TSB[:n, 0:1], op0=Alu.add, op1=Alu.mult)
            y1 = sb.tile([128, 1], f32, tag="y1", bufs=2)
            nc.vector.scalar_tensor_tensor(
                out=y1[:n], in0=crd[:n, 1:2], scalar=h05[:n, 0:1],
                in1=TSB[:n, 1:2], op0=Alu.subtract, op1=Alu.mult)
            y2 = sb.tile([128, 1], f32, tag="y2", bufs=2)
            nc.vector.scalar_tensor_tensor(
                out=y2[:n], in0=crd[:n, 1:2], scalar=h05[:n, 0:1],
                in1=TSB[:n, 1:2], op0=Alu.add, op1=Alu.mult)

            XT = sb.tile([128, 32], f32, tag="XT", bufs=2)
            tmp = sb.tile([128, 32], f32, tag="tmp", bufs=2)
            nc.vector.tensor_scalar(out=tmp[:n], in0=g16[:n],
                                    scalar1=x2[:n, 0:1], scalar2=None,
                                    op0=Alu.is_lt)
            nc.vector.scalar_tensor_tensor(
                out=XT[:n], in0=g16[:n], scalar=x1[:n, 0:1], in1=tmp[:n],
                op0=Alu.is_gt, op1=Alu.mult)
            # fold top-k selection into the X mask (on DVE; waits for sel)
            nc.vector.tensor_tensor(out=XT[:n], in0=XT[:n],
                                    in1=sel[:n, 0:1].to_broadcast([n, 32]),
                                    op=Alu.mult)

            YT = sb.tile([128, 32], f32, tag="YT", bufs=2)
            tmp2 = sb.tile([128, 32], f32, tag="tmp2", bufs=2)
            nc.vector.tensor_scalar(out=tmp2[:n], in0=g16[:n],
                                    scalar1=y2[:n, 0:1], scalar2=None,
                                    op0=Alu.is_lt)
            nc.vector.scalar_tensor_tensor(
                out=YT[:n], in0=g16[:n], scalar=y1[:n, 0:1], in1=tmp2[:n],
                op0=Alu.is_gt, op1=Alu.mult)

            # S[i, j] += sum_q YT[q, i] * XT[q, j]
            nc.tensor.matmul(out=S32[:], lhsT=YT[:n], rhs=XT[:n],
                             start=(k == 0), stop=(k == len(CHUNKS) - 1))

        # ---------------- keep-mask and destination indices ----------------
        PADF = sb.tile([32, 32], f32, name="PADF")
        nc.vector.tensor_copy(out=PADF[:32], in_=PAD8[:32])
        M = sb.tile([32, 32], f32, name="M")
        nc.vector.scalar_tensor_tensor(
            out=M[:32], in0=S32[:], scalar=0.0, in1=PADF[:32],
            op0=Alu.is_equal, op1=Alu.max)

        rsum = sb.tile([32, 1], f32, name="rsum")
        nc.vector.tensor_reduce(rsum[:32], M[:32, :], axis=AX.X, op=Alu.add)
        MT = sb.tile([32, 32], f32, name="MT")
        nc.vector.transpose(MT[:32], M[:32])

        e_ps = ps.tile([32, 32], f32, tag="eps")
        nc.tensor.matmul(out=e_ps[:], lhsT=MT[:32], rhs=T32[:32],
                         start=True, stop=True)
        roff_ps = ps.tile([32, 1], f32, tag="roff")
        nc.tensor.matmul(out=roff_ps[:], lhsT=T32[:32], rhs=rsum[:32],
                         start=True, stop=True)
        roff = sb.tile([32, 1], f32, name="roff")
        nc.vector.tensor_copy(out=roff[:32], in_=roff_ps[:])

        # dest = (e + roff + 4000) - 4000*M : kept rows -> d, dropped -> >= 4000 (OOB)
        A = sb.tile([32, 32], f32, name="A")
        nc.vector.tensor_scalar(out=A[:32], in0=e_ps[:],
                                scalar1=roff[:32, 0:1], scalar2=4000.0,
                                op0=Alu.add, op1=Alu.add)
        DSTF = sb.tile([32, 32], f32, name="DSTF")
        nc.vector.scalar_tensor_tensor(
            out=DSTF[:32], in0=M[:32], scalar=-4000.0, in1=A[:32],
            op0=Alu.mult, op1=Alu.add)
        DI = sb.tile([32, 32], i32, name="DI")
        nc.vector.tensor_copy(out=DI[:32], in_=DSTF[:32])

        # roundtrip through DRAM to regroup (32i, 32j) -> (128p, 8t)
        dest_d = dr.tile([HW], i32, name="dest_d")
        nc.scalar.dma_start(out=dest_d[:].rearrange("(i j) -> i j", j=32),
                            in_=DI[:32])
        DOFF = sb.tile([128, NT], i32, name="DOFF")
        nc.scalar.dma_start(out=DOFF[:],
                            in_=dest_d[:].rearrange("(t p) -> p t", p=128))

        # ------- transpose x / pos into interleaved (token, 2*channel) -------
        # XPT_all[:, 256t:256t+128] = x columns tile t transposed,
        # XPT_all[:, 256t+128:256t+256] = pos columns tile t transposed.
        # Row g of the combined (1024, 256) output = [x_row(g) | pos_row(g)].
        XPT_all = sb.tile([128, 2 * HW], f32, name="XPT_all")
        for t in range(NT):
            cols = slice(128 * t, 128 * (t + 1))
            xp = ps.tile([128, 128], f32, tag="xp", bufs=2)
            nc.tensor.transpose(out=xp[:], in_=XH[:, cols], identity=ident[:])
            nc.vector.tensor_copy(out=XPT_all[:, 256 * t:256 * t + 128], in_=xp[:])
            pp = ps.tile([128, 128], f32, tag="xp", bufs=2)
            nc.tensor.transpose(out=pp[:], in_=PH[:, cols], identity=ident[:])
            nc.vector.tensor_copy(out=XPT_all[:, 256 * t + 128:256 * t + 256],
                                  in_=pp[:])

        # ---------------- scatter rows to their destinations ----------------
        # HW dynamic DMA consumes ONE offset per partition, so issue one
        # indirect DMA per 128-row tile: offsets (128,1), rows of 1 KiB.
        # The 8 scatters write disjoint rows and are FIFO-ordered on the same
        # SWDGE queue; drop the tracker's WAW edges so they pipeline instead
        # of waiting for each other's completion semaphore.
        for t in range(NT):
            nc.gpsimd.indirect_dma_start(
                out=io["skp"],
                out_offset=bass.IndirectOffsetOnAxis(
                    ap=DOFF[:, t:t + 1], axis=0),
                in_=XPT_all[:, 256 * t:256 * (t + 1)],
                in_offset=None,
                bounds_check=HW - 1,
                oob_is_err=False)
            tc.dep_state.clear_tensor_accesses("skp")

        if "dbg" in io:
            nc.sync.dma_start(out=io["dbg_crow"], in_=CROW[:1, :])
            nc.sync.dma_start(out=io["dbg_cbc"], in_=CBC[:, :])
            nc.sync.dma_start(out=io["dbg_m"], in_=M[:32])
            nc.sync.dma_start(out=io["dbg_dstf"], in_=DSTF[:32])
            nc.sync.dma_start(out=io["dbg_doff"], in_=DOFF[:])
            nc.sync.dma_start(out=io["dbg_xt"], in_=XPT_all[:, :HW])


def _build(dbg=False):
    if "nc" in _cache:
        return _cache["nc"]
    from concourse import bacc, mybir, tile
    import concourse.bass as bass

    dt = mybir.dt
    nc = bacc.Bacc("TRN2", target_bir_lowering=False, debug=False,
                   enable_asserts=False, num_devices=NCORES)

    io = {
        "xh": nc.dram_tensor("xh", [CH, HW], dt.float32, kind="ExternalInput").ap(),
        "ph": nc.dram_tensor("ph", [CH, HW], dt.float32, kind="ExternalInput").ap(),
        "cls": nc.dram_tensor("cls", [NQ, NCLS], dt.float32, kind="ExternalInput").ap(),
        "crd": nc.dram_tensor("crd", [NQ, 4], dt.float32, kind="ExternalInput").ap(),
        "tsz": nc.dram_tensor("tsz", [1, 2], dt.int32, kind="ExternalInput").ap(),
        "pmask": nc.dram_tensor("pmask", [H, W], dt.uint8, kind="ExternalInput").ap(),
        "skp": nc.dram_tensor("skp", [HW, 2 * CH], dt.float32,
                              kind="ExternalOutput").ap(),
    }
    if dbg:
        io["dbg"] = True
        io["dbg_crow"] = nc.dram_tensor("dbg_crow", [1, NQ], dt.float32, kind="ExternalOutput").ap()
        io["dbg_cbc"] = nc.dram_tensor("dbg_cbc", [128, NQ], dt.float32, kind="ExternalOutput").ap()
        io["dbg_m"] = nc.dram_tensor("dbg_m", [32, 32], dt.float32, kind="ExternalOutput").ap()
        io["dbg_dstf"] = nc.dram_tensor("dbg_dstf", [32, 32], dt.float32, kind="ExternalOutput").ap()
        io["dbg_doff"] = nc.dram_tensor("dbg_doff", [128, NT], dt.int32, kind="ExternalOutput").ap()
        io["dbg_xt"] = nc.dram_tensor("dbg_xt", [128, HW], dt.float32, kind="ExternalOutput").ap()
    _cache["io"] = io

    with tile.TileContext(nc) as tc:
        _emit(tc, bass, mybir)
    nc.compile()
    _cache["nc"] = nc
    return nc


def _in_maps(x, pos_embed, mask_u8, outputs_coord, outputs_class, its):
    maps = []
    for core in range(NCORES):
        b, h = divmod(core, 2)
        maps.append({
            "xh": np.ascontiguousarray(x[b].reshape(C, HW)[h * CH:(h + 1) * CH]),
            "ph": np.ascontiguousarray(
                pos_embed[b].reshape(C, HW)[h * CH:(h + 1) * CH]),
            "cls": np.ascontiguousarray(outputs_class[b]),
            "crd": np.ascontiguousarray(outputs_coord[b]),
            "tsz": np.ascontiguousarray(its[b:b + 1]),
            "pmask": np.ascontiguousarray(mask_u8[b]),
        })
    return maps


def kernel(x, pos_embed, mask, outputs_coord, outputs_class,
           img_true_sizes, batched_h, batched_w, _trace=False):
    assert int(batched_h) == 512 and int(batched_w) == 512

    x = np.asarray(x, dtype=np.float32)
    pos_embed = np.asarray(pos_embed, dtype=np.float32)
    mask_u8 = np.asarray(mask).astype(np.uint8)
    outputs_coord = np.asarray(outputs_coord, dtype=np.float32)
    outputs_class = np.asarray(outputs_class, dtype=np.float32)
    its = np.asarray(img_true_sizes, dtype=np.int32)

    nc = _build()
    from concourse import bass_utils
    res = bass_utils.run_bass_kernel_spmd(
        nc, _in_maps(x, pos_embed, mask_u8, outputs_coord, outputs_class, its),
        core_ids=list(range(NCORES)), trace=_trace)

    sk = np.empty((HW, BS, C), np.float32)
    sp = np.empty((HW, BS, C), np.float32)
    for core in range(NCORES):
        b, h = divmod(core, 2)
        skp = res.results[core]["skp"]
        sk[:, b, h * CH:(h + 1) * CH] = skp[:, :CH]
        sp[:, b, h * CH:(h + 1) * CH] = skp[:, CH:]
    if _trace:
        kernel.last_results = res
    return sk, sp

